# revision 1
# baseline (speedup 1.0000x reference)
"""Tensor-parallel MHSA (RoPE + causal attention) for 8 TRN2 NeuronCores.

Sharding: 8-way tensor-parallel over heads (16 heads -> 2 per core).
Each core computes q/k/v projections for its 2 heads (column-parallel),
RoPE, causal attention, and a row-parallel slice of the output projection,
producing a full-shape partial y^T; the host sums the 8 partials.

Layout: activations feature-major ([feature, token]) so every matmul
contracts over the partition dim.  Scores are computed transposed
(S^T[m, l]) so softmax sums become ones-vector matmuls on the PE and no
P-transposes are needed for A@V.  exp runs without max-subtraction
(scores are O(4) for this problem's 0.02-scaled weights — safe in fp32).
All matmuls in fp32r (full PE rate at free-dim>=256, ~1e-4 precision).
"""
import sys
sys.path.insert(0, "/opt/trn_rl_repo")
import numpy as np

B, L, E = 2, 2048, 2048
HEADS = 16
HD = 128
BASE = 10000.0
NCORES = 8
HPC = HEADS // NCORES      # heads per core = 2
COLS = HPC * HD            # 256 columns of Wq/Wk/Wv per core
KT = E // 128              # 16 k-tiles
LC = L // 512              # 4 l-chunks (attention / out-proj)
SC = L // 256              # 8 sub-chunks (qkv projection)
NEG = -1.0e9


def _build_program():
    import concourse.bass as bass
    import concourse.mybir as mybir
    import concourse.tile as tile
    from concourse import bacc
    from concourse.alu_op_type import AluOpType

    F32 = mybir.dt.float32
    F32R = mybir.dt.float32r
    Exp = mybir.ActivationFunctionType.Exp

    nc = bacc.Bacc()
    xT_d = nc.declare_dram_parameter("xT", [B, E, L], F32R, isOutput=False)
    wq_d = nc.declare_dram_parameter("wq", [E, COLS], F32R, isOutput=False)
    wk_d = nc.declare_dram_parameter("wk", [E, COLS], F32R, isOutput=False)
    wv_d = nc.declare_dram_parameter("wv", [E, COLS], F32R, isOutput=False)
    wo_d = nc.declare_dram_parameter("wo", [COLS, E], F32R, isOutput=False)
    bq_d = nc.declare_dram_parameter("bq", [1, COLS], F32R, isOutput=False)
    bk_d = nc.declare_dram_parameter("bk", [1, COLS], F32R, isOutput=False)
    bv_d = nc.declare_dram_parameter("bv", [1, COLS], F32R, isOutput=False)
    cos_d = nc.declare_dram_parameter("cosf", [64, L], F32, isOutput=False)
    sin_d = nc.declare_dram_parameter("sinf", [64, L], F32, isOutput=False)
    mask_d = nc.declare_dram_parameter("mask", [128, 128], F32, isOutput=False)
    ones_d = nc.declare_dram_parameter("ones", [128, 256], F32R, isOutput=False)
    y_d = nc.declare_dram_parameter("yT", [B, E, L], F32, isOutputTrue := True)

    with nc.allow_low_precision(reason="fp32r matmuls"), \
         tile.TileContext(nc) as tc:
        with (
            tc.tile_pool(name="fixed", bufs=1) as fixed,
            tc.tile_pool(name="qkv", bufs=1) as qkvp,
            tc.tile_pool(name="xs", bufs=2) as xs,
            tc.tile_pool(name="pt", bufs=3) as ptp,
            tc.tile_pool(name="yst", bufs=3) as yst,
            tc.tile_pool(name="small", bufs=2) as smallp,
        ):
            wq_sb = fixed.tile([128, KT, COLS], F32R, name="wq", tag="wq")
            nc.sync.dma_start(
                out=wq_sb, in_=wq_d[:, :].rearrange("(kt p) c -> p kt c", p=128))
            wk_sb = fixed.tile([128, KT, COLS], F32R, name="wk", tag="wk")
            nc.sync.dma_start(
                out=wk_sb, in_=wk_d[:, :].rearrange("(kt p) c -> p kt c", p=128))
            wv_sb = fixed.tile([128, KT, COLS], F32R, name="wv", tag="wv")
            nc.sync.dma_start(
                out=wv_sb, in_=wv_d[:, :].rearrange("(kt p) c -> p kt c", p=128))
            ones = fixed.tile([128, 256], F32R, name="ones", tag="ones")
            nc.sync.dma_start(out=ones, in_=ones_d[:, :])
            cos_sb = fixed.tile([64, L], F32, name="cos", tag="cos")
            nc.sync.dma_start(out=cos_sb, in_=cos_d[:, :])
            sin_sb = fixed.tile([64, L], F32, name="sin", tag="sin")
            nc.sync.dma_start(out=sin_sb, in_=sin_d[:, :])
            mask_sb = fixed.tile([128, 128], F32, name="mask", tag="mask")
            nc.sync.dma_start(out=mask_sb, in_=mask_d[:, :])
            bq_sb = fixed.tile([1, COLS], F32R, name="bq", tag="bq")
            nc.sync.dma_start(out=bq_sb, in_=bq_d[:, :])
            bk_sb = fixed.tile([1, COLS], F32R, name="bk", tag="bk")
            nc.sync.dma_start(out=bk_sb, in_=bk_d[:, :])
            bv_sb = fixed.tile([1, COLS], F32R, name="bv", tag="bv")
            nc.sync.dma_start(out=bv_sb, in_=bv_d[:, :])

            wo_sb = fixed.tile([128, HPC, E], F32R, name="wo", tag="wo")
            nc.sync.dma_start(
                out=wo_sb, in_=wo_d[:, :].rearrange("(h p) e -> p h e", p=128))

            qT = [qkvp.tile([128, L], F32R, name=f"qT{h}", tag=f"qT{h}") for h in range(HPC)]
            kT = [qkvp.tile([128, L], F32R, name=f"kT{h}", tag=f"kT{h}") for h in range(HPC)]
            oT = [qkvp.tile([128, L], F32R, name=f"oT{h}", tag=f"oT{h}") for h in range(HPC)]
            vv = qkvp.tile([128, 16, COLS], F32R, name="vv", tag="vv")  # [m-part, mb, cols]

            for b in range(B):
                # ---------- QKV projection: 256-wide sub-chunks, k-contiguous ----
                with tc.tile_pool(name=f"psq{b}", bufs=1, space="PSUM") as psq:
                    for sc in range(SC):
                        xt = xs.tile([128, KT, 256], F32R, name="xt", tag="xt")
                        nc.sync.dma_start(
                            out=xt,
                            in_=xT_d[b, :, sc * 256:(sc + 1) * 256]
                            .rearrange("(kt p) n -> p kt n", p=128))
                        qps = [psq.tile([128, 256], F32, name=f"qps{h}", tag=f"qps{h}") for h in range(HPC)]
                        kps = [psq.tile([128, 256], F32, name=f"kps{h}", tag=f"kps{h}") for h in range(HPC)]
                        vps = [psq.tile([128, COLS], F32, name=f"vps{i}", tag=f"vps{i}") for i in range(2)]
                        for k in range(KT):
                            for h in range(HPC):
                                nc.tensor.matmul(
                                    qps[h], lhsT=wq_sb[:, k, h * 128:(h + 1) * 128],
                                    rhs=xt[:, k, :], start=(k == 0), stop=False)
                                nc.tensor.matmul(
                                    kps[h], lhsT=wk_sb[:, k, h * 128:(h + 1) * 128],
                                    rhs=xt[:, k, :], start=(k == 0), stop=False)
                            for i in range(2):
                                nc.tensor.matmul(
                                    vps[i], lhsT=xt[:, k, i * 128:(i + 1) * 128],
                                    rhs=wv_sb[:, k, :], start=(k == 0), stop=False)
                        for h in range(HPC):
                            nc.tensor.matmul(
                                qps[h], lhsT=bq_sb[0:1, h * 128:(h + 1) * 128],
                                rhs=ones[0:1, :], start=False, stop=True)
                            nc.tensor.matmul(
                                kps[h], lhsT=bk_sb[0:1, h * 128:(h + 1) * 128],
                                rhs=ones[0:1, :], start=False, stop=True)
                        for i in range(2):
                            nc.tensor.matmul(
                                vps[i], lhsT=ones[0:1, 0:128],
                                rhs=bv_sb[0:1, :], start=False, stop=True)
                            nc.scalar.copy(out=vv[:, sc * 2 + i, :], in_=vps[i])
                        # RoPE (rotate halves) on q/k sub-chunks, psum -> sbuf
                        sl = slice(sc * 256, (sc + 1) * 256)
                        for h in range(HPC):
                            for ps, dst in ((qps[h], qT[h]), (kps[h], kT[h])):
                                t1 = smallp.tile([128, 256], F32, name="ropet1", tag="ropet1")
                                nc.vector.scalar_tensor_tensor(
                                    out=t1[0:64, :], in0=ps[64:128, :], scalar=-1.0,
                                    in1=sin_sb[:, sl], op0=AluOpType.mult,
                                    op1=AluOpType.mult)
                                nc.vector.tensor_mul(
                                    t1[64:128, :], ps[0:64, :], sin_sb[:, sl])
                                t2 = smallp.tile([128, 256], F32, name="ropet2", tag="ropet2")
                                nc.vector.tensor_mul(t2[0:64, :], ps[0:64, :], cos_sb[:, sl])
                                nc.vector.tensor_mul(t2[64:128, :], ps[64:128, :], cos_sb[:, sl])
                                nc.vector.tensor_add(dst[:, sl], t1, t2)

                # ---------- attention per head (S^T layout, causal) ----------
                with (
                    tc.tile_pool(name=f"psa{b}", bufs=1, space="PSUM") as psa,
                    tc.tile_pool(name=f"pss{b}", bufs=2, space="PSUM") as pss,
                ):
                    for h in range(HPC):
                        for lc in range(LC):
                            av = psa.tile([128, 512], F32, name="av", tag="av")
                            rs = psa.tile([1, 512], F32, name="rs", tag="rs")
                            for mb in range(4 * lc + 4):
                                l0 = max(lc * 512, mb * 128)
                                npr = lc * 512 + 512 - l0
                                c0 = l0 - lc * 512
                                st = pss.tile([128, 512], F32, name="st", tag="st")
                                nc.tensor.matmul(
                                    st[:, 0:npr], lhsT=kT[h][:, mb * 128:(mb + 1) * 128],
                                    rhs=qT[h][:, l0:l0 + npr], start=True, stop=True)
                                if mb >= 4 * lc:  # diagonal block: causal mask
                                    nc.vector.tensor_add(
                                        st[:, 0:128], st[:, 0:128], mask_sb)
                                pt = ptp.tile([128, 512], F32R, name="pt", tag="pt")
                                nc.scalar.activation(
                                    out=pt[:, 0:npr], in_=st[:, 0:npr], func=Exp)
                                nc.tensor.matmul(
                                    av[:, c0:512],
                                    lhsT=vv[:, mb, h * 128:(h + 1) * 128],
                                    rhs=pt[:, 0:npr], start=(mb == 0),
                                    stop=(mb == 4 * lc + 3))
                                nc.tensor.matmul(
                                    rs[0:1, c0:512], lhsT=ones[:, 0:1],
                                    rhs=pt[:, 0:npr], start=(mb == 0),
                                    stop=(mb == 4 * lc + 3))
                            rec = smallp.tile([1, 512], F32R, name="rec", tag="rec")
                            nc.vector.reciprocal(out=rec, in_=rs[0:1, :])
                            bc = psa.tile([128, 512], F32, name="bc", tag="bc")
                            nc.tensor.matmul(bc, lhsT=ones[0:1, 0:128], rhs=rec,
                                             start=True, stop=True)
                            bcs = smallp.tile([128, 512], F32, name="bcs", tag="bcs")
                            nc.scalar.copy(out=bcs, in_=bc)
                            nc.vector.tensor_mul(
                                oT[h][:, lc * 512:(lc + 1) * 512], av, bcs)

                # ---------- output projection (row-parallel partial) ----------
                with tc.tile_pool(name=f"psy{b}", bufs=3, space="PSUM") as psy:
                    for eb in range(KT):
                        for lc in range(LC):
                            yp = psy.tile([128, 512], F32, name="yp", tag="yp")
                            for h in range(HPC):
                                nc.tensor.matmul(
                                    yp, lhsT=wo_sb[:, h, eb * 128:(eb + 1) * 128],
                                    rhs=oT[h][:, lc * 512:(lc + 1) * 512],
                                    start=(h == 0), stop=(h == HPC - 1))
                            ys = yst.tile([128, 512], F32, name="ys", tag="ys")
                            if (eb + lc) % 2 == 0:
                                nc.scalar.copy(out=ys, in_=yp)
                            else:
                                nc.vector.tensor_copy(ys, yp)
                            nc.sync.dma_start(
                                out=y_d[b, eb * 128:(eb + 1) * 128,
                                        lc * 512:(lc + 1) * 512],
                                in_=ys)
    nc.compile()
    return nc


_NC_CACHE = None


def kernel(x, Wq, bq, Wk, bk, Wv, bv, Wo, bo):
    global _NC_CACHE
    from concourse.bass_utils import run_bass_kernel_spmd

    x = np.asarray(x, np.float32)
    scale = HD ** (-0.5)

    inv = 1.0 / (BASE ** (np.arange(0, HD, 2, dtype=np.float32) / HD))
    fr = np.outer(inv, np.arange(L, dtype=np.float32))  # [64, L]
    cosf = np.cos(fr).astype(np.float32)
    sinf = np.sin(fr).astype(np.float32)
    mask = np.where(np.arange(128)[:, None] <= np.arange(128)[None, :],
                    0.0, NEG).astype(np.float32)

    xT = np.ascontiguousarray(np.transpose(x, (0, 2, 1)))  # [B, E, L]

    in_maps = []
    for c in range(NCORES):
        cols = slice(c * COLS, (c + 1) * COLS)
        bq_c = (np.asarray(bq)[cols] * scale).astype(np.float32)[None, :]
        bk_c = np.asarray(bk, np.float32)[cols][None, :]
        bv_c = np.asarray(bv, np.float32)[cols][None, :]
        in_maps.append({
            "xT": xT,
            "wq": np.ascontiguousarray(np.asarray(Wq, np.float32)[:, cols]) * scale,
            "wk": np.ascontiguousarray(np.asarray(Wk, np.float32)[:, cols]),
            "wv": np.ascontiguousarray(np.asarray(Wv, np.float32)[:, cols]),
            "wo": np.ascontiguousarray(np.asarray(Wo, np.float32)[cols, :]),
            "bq": bq_c, "bk": bk_c, "bv": bv_c,
            "cosf": cosf,
            "sinf": sinf,
            "mask": mask,
            "ones": np.ones((128, 256), np.float32),
        })

    if _NC_CACHE is None:
        _NC_CACHE = _build_program()
    import os
    if os.environ.get("BASS_PROFILE"):
        res = run_bass_kernel_spmd(_NC_CACHE, in_maps, list(range(NCORES)),
                                   trace=True, tmpdir="/tmp/mhsa_prof")
        print(f"HW exec time: {res.exec_time_ns} ns")
    else:
        res = run_bass_kernel_spmd(_NC_CACHE, in_maps, list(range(NCORES)))
    acc = np.zeros((B, E, L), np.float64)
    for c in range(NCORES):
        acc += res.results[c]["yT"].astype(np.float32)
    y = np.transpose(acc, (0, 2, 1)).astype(np.float32) + np.asarray(bo, np.float32)
    return y



# revision 2
# speedup vs baseline: 1.0232x; 1.0232x over previous
"""Tensor-parallel MHSA (RoPE + causal attention) for 8 TRN2 NeuronCores, v2.

Sharding: 8-way tensor-parallel over heads (16 heads -> 2 per core), both
batches on every core.  Each core computes q/k/v projections for its 2 heads
(column-parallel), RoPE, causal attention, and a row-parallel slice of the
output projection, producing a full-shape partial y^T in bf16; the host sums
the 8 partials and adds bo + Wo^T bv (the v-bias folds out of attention since
softmax rows sum to 1).

Key structure:
- all matmuls bf16 (same PE rate as fp32r, half the DMA/SBUF of fp32)
- few, large DMAs (HWDGE issue overhead is ~625ns each): weights land in one
  pre-transposed [128, KT, COLS] transfer each, x in 1MB k-pair tiles,
  y in [128, 4eb, 512] groups, constants packed/memset
- QKV runs as three k-outer phases (q, k, v) over 8 single-bank PSUM
  accumulation chains, so each arriving x k-pair immediately feeds 32
  matmuls (hides the DMA ramp) and each stationary weight is reused 8x
  (amortizes LDWEIGHTS)
- each PSUM bank holds exactly ONE accumulation group (zero-region rule):
  both heads' chains in a bank share one start/stop pair
- q/k biases applied during PSUM evacuation (ACT Identity+bias / DVE
  tensor_scalar_add); v bias folds into the host-side output bias
- RoPE on half-rows with bf16 2x DVE tensor_tensor (sign folded into table)
- causal mask added by a tiny PE matmul (maskT^T @ I) inside the scores
  accumulation group
- attention: S^T layout, lookahead-2 block pipeline, exp on ACT -> bf16;
  out-proj for chunk lc-1 emitted inside chunk lc's first head so the PE
  never waits on the softmax-normalize chain
"""
import sys
sys.path.insert(0, "/opt/trn_rl_repo")
import numpy as np

B, L, E = 2, 2048, 2048
HEADS = 16
HD = 128
BASE = 10000.0
NCORES = 8
HPC = HEADS // NCORES      # heads per core = 2
COLS = HPC * HD            # 256 columns of Wq/Wk/Wv per core
KT = E // 128              # 16 k-tiles
LC = L // 512              # 4 l-chunks (attention / out-proj)
SC = L // 256              # 8 sub-chunks (qkv projection)
NEG = -1.0e9


def _build_program():
    import concourse.bass as bass
    import concourse.mybir as mybir
    import concourse.tile as tile
    from concourse import bacc

    F32 = mybir.dt.float32
    F32R = mybir.dt.float32r
    BF16 = mybir.dt.bfloat16
    Exp = mybir.ActivationFunctionType.Exp

    nc = bacc.Bacc()
    x_d = nc.declare_dram_parameter("xkt", [B, KT, 128, L], BF16, isOutput=False)
    wq_d = nc.declare_dram_parameter("wq", [128, KT, COLS], BF16, isOutput=False)
    wk_d = nc.declare_dram_parameter("wk", [128, KT, COLS], BF16, isOutput=False)
    wv_d = nc.declare_dram_parameter("wv", [128, KT, COLS], BF16, isOutput=False)
    wo_d = nc.declare_dram_parameter("wo", [128, HPC, E], BF16, isOutput=False)
    bias_d = nc.declare_dram_parameter("bias2", [128, 8], F32, isOutput=False)
    tabs_d = nc.declare_dram_parameter("tabs", [128, 2, L], BF16, isOutput=False)
    mi_d = nc.declare_dram_parameter("mi", [128, 256], BF16, isOutput=False)
    y_d = nc.declare_dram_parameter("yT", [B, E, L], BF16, isOutput=True)

    with nc.allow_low_precision(reason="bf16 matmuls"), \
         tile.TileContext(nc) as tc:
        with (
            tc.tile_pool(name="fixed", bufs=1) as fixed,
            tc.tile_pool(name="xs", bufs=1) as xs,
            tc.tile_pool(name="qkv", bufs=1) as qkvp,
            tc.tile_pool(name="rope", bufs=2) as rp,
            tc.tile_pool(name="bc", bufs=1) as bcp,
            tc.tile_pool(name="pt", bufs=4) as ptp,
            tc.tile_pool(name="ot", bufs=2) as otp,
            tc.tile_pool(name="yst", bufs=3) as yst,
            tc.tile_pool(name="small", bufs=2) as smallp,
        ):
            # DMA order = consumption order: wq, then x pairs (the q phase
            # tracks x arrival), then wk/wv, then late-use constants.
            wq_sb = fixed.tile([128, KT, COLS], BF16, name="wq", tag="wq")
            nc.sync.dma_start(out=wq_sb, in_=wq_d[:, :, :])
            xk = [xs.tile([128, 2, L], BF16, name=f"xk{g}", tag=f"xk{g}")
                  for g in range(KT // 2)]
            for g in range(KT // 2):
                nc.sync.dma_start(
                    out=xk[g],
                    in_=x_d[0, 2 * g:2 * g + 2].rearrange("k p l -> p k l"))
            wk_sb = fixed.tile([128, KT, COLS], BF16, name="wk", tag="wk")
            nc.sync.dma_start(out=wk_sb, in_=wk_d[:, :, :])
            wv_sb = fixed.tile([128, KT, COLS], BF16, name="wv", tag="wv")
            nc.sync.dma_start(out=wv_sb, in_=wv_d[:, :, :])
            bias_sb = fixed.tile([128, 8], F32, name="bias2", tag="bias2")
            nc.sync.dma_start(out=bias_sb, in_=bias_d[:, :])
            tabs_sb = fixed.tile([128, 2, L], BF16, name="tabs", tag="tabs")
            nc.sync.dma_start(out=tabs_sb, in_=tabs_d[:, :, :])
            mi_sb = fixed.tile([128, 256], BF16, name="mi", tag="mi")
            nc.sync.dma_start(out=mi_sb, in_=mi_d[:, :])
            wo_sb = fixed.tile([128, HPC, E], BF16, name="wo", tag="wo")
            nc.sync.dma_start(out=wo_sb, in_=wo_d[:, :, :])

            cos_sb = tabs_sb[:, 0, :]
            sin_sb = tabs_sb[:, 1, :]
            mask_sb = mi_sb[:, 0:128]
            id_sb = mi_sb[:, 128:256]
            ones_sb = fixed.tile([128, 1], BF16, name="onesb", tag="onesb")
            nc.vector.memset(ones_sb, 1.0)

            def xkt_ap(k):
                return xk[k // 2][:, k % 2, :]

            for b in range(B):
                if b > 0:
                    for g in range(KT // 2):
                        nc.sync.dma_start(
                            out=xk[g],
                            in_=x_d[b, 2 * g:2 * g + 2]
                            .rearrange("k p l -> p k l"))

                qT = qkvp.tile([128, HPC, L], BF16, name="qT", tag="qT")
                kT = qkvp.tile([128, HPC, L], BF16, name="kT", tag="kT")
                qraw = qkvp.tile([128, HPC, L], BF16, name="qraw", tag="qraw")
                kraw = qkvp.tile([128, HPC, L], BF16, name="kraw", tag="kraw")
                qsw = qkvp.tile([128, HPC, L], BF16, name="qsw", tag="qsw")
                ksw = qkvp.tile([128, HPC, L], BF16, name="ksw", tag="ksw")
                vv = qkvp.tile([128, 16, COLS], BF16, name="vv", tag="vv")

                def rope_chunk(hc):
                    ch = slice(hc * 1024, hc * 1024 + 1024)
                    for h in range(HPC):
                        for src, ssw, dst in ((qraw, qsw, qT), (kraw, ksw, kT)):
                            t1 = rp.tile([128, 1024], BF16, name="t1", tag="t1")
                            nc.vector.tensor_mul(t1, ssw[:, h, ch], sin_sb[:, ch])
                            t2 = rp.tile([128, 1024], BF16, name="t2", tag="t2")
                            nc.vector.tensor_mul(t2, src[:, h, ch], cos_sb[:, ch])
                            nc.vector.tensor_add(dst[:, h, ch], t1, t2)

                # ---------- QKV projection over 8 PSUM banks ----------
                # q phase is k-outer so each arriving x k-pair feeds all 8
                # chains (tracks the DMA ramp); k and v phases run sc-outer
                # (x already resident) so their evacuations spread across the
                # phase instead of bunching at its end.
                with tc.tile_pool(name=f"psq{b}", bufs=1, space="PSUM") as psq:
                    qps = [psq.tile([128, HPC, 256], F32,
                                    name=f"ps{sc}", tag=f"ps{sc}")
                           for sc in range(SC)]
                    for k in range(KT):
                        for h in range(HPC):
                            for sc in range(SC):
                                nc.tensor.matmul(
                                    qps[sc][:, h, :],
                                    lhsT=wq_sb[:, k, h * 128:(h + 1) * 128],
                                    rhs=xkt_ap(k)[:, sc * 256:(sc + 1) * 256],
                                    start=(k == 0 and h == 0),
                                    stop=(k == KT - 1 and h == HPC - 1))
                    for sc in range(SC):
                        sl = slice(sc * 256, (sc + 1) * 256)
                        nc.scalar.add(out=qraw[:, 0, sl], in_=qps[sc][:, 0, :],
                                      add=bias_sb[:, 0:1])
                        nc.vector.tensor_scalar_add(
                            out=qraw[:, 1, sl], in0=qps[sc][:, 1, :],
                            scalar1=bias_sb[:, 1:2])
                        for hh, half in ((0, slice(0, 64)), (0, slice(64, 128)),
                                         (1, slice(0, 64)), (1, slice(64, 128))):
                            osl = slice(64 - half.start, 128 - half.start)
                            if hh == 0:
                                nc.scalar.add(
                                    out=qsw[half, 0, sl],
                                    in_=qps[sc][osl, 0, :],
                                    add=bias_sb[half, 4:5])
                            else:
                                nc.vector.tensor_scalar_add(
                                    out=qsw[half, 1, sl],
                                    in0=qps[sc][osl, 1, :],
                                    scalar1=bias_sb[half, 5:6])

                    for sc in range(SC):
                        kps = psq.tile([128, HPC, 256], F32,
                                       name=f"ps{sc}", tag=f"ps{sc}")
                        sl = slice(sc * 256, (sc + 1) * 256)
                        for k in range(KT):
                            for h in range(HPC):
                                nc.tensor.matmul(
                                    kps[:, h, :],
                                    lhsT=wk_sb[:, k, h * 128:(h + 1) * 128],
                                    rhs=xkt_ap(k)[:, sl],
                                    start=(k == 0 and h == 0),
                                    stop=(k == KT - 1 and h == HPC - 1))
                        nc.scalar.add(out=kraw[:, 0, sl], in_=kps[:, 0, :],
                                      add=bias_sb[:, 2:3])
                        nc.vector.tensor_scalar_add(
                            out=kraw[:, 1, sl], in0=kps[:, 1, :],
                            scalar1=bias_sb[:, 3:4])
                        for hh, half in ((0, slice(0, 64)), (0, slice(64, 128)),
                                         (1, slice(0, 64)), (1, slice(64, 128))):
                            osl = slice(64 - half.start, 128 - half.start)
                            if hh == 0:
                                nc.scalar.add(
                                    out=ksw[half, 0, sl],
                                    in_=kps[osl, 0, :],
                                    add=bias_sb[half, 6:7])
                            else:
                                nc.vector.tensor_scalar_add(
                                    out=ksw[half, 1, sl],
                                    in0=kps[osl, 1, :],
                                    scalar1=bias_sb[half, 7:8])
                        if sc == 3:
                            rope_chunk(0)
                    rope_chunk(1)

                    # v phase: V in token-major (x slices stationary)
                    for sc in range(SC):
                        vps = psq.tile([128, 2, 256], F32,
                                       name=f"ps{sc}", tag=f"ps{sc}")
                        for k in range(KT):
                            for i in range(2):
                                nc.tensor.matmul(
                                    vps[:, i, :],
                                    lhsT=xkt_ap(k)[:, sc * 256 + i * 128:
                                                   sc * 256 + (i + 1) * 128],
                                    rhs=wv_sb[:, k, :],
                                    start=(k == 0 and i == 0),
                                    stop=(k == KT - 1 and i == 1))
                        nc.scalar.copy(out=vv[:, 2 * sc:2 * sc + 2, :],
                                       in_=vps)

                # ---------- causal attention + interleaved out-proj ----------
                oTs = {}

                def out_proj_group(lc, ebg):
                    if True:
                        ys = yst.tile([128, 4, 512], BF16, name="ys", tag="ys")
                        for i in range(4):
                            eb = ebg * 4 + i
                            yp = psy.tile([128, 512], F32, name="yp", tag="yp")
                            for h in range(HPC):
                                nc.tensor.matmul(
                                    yp,
                                    lhsT=wo_sb[:, h, eb * 128:(eb + 1) * 128],
                                    rhs=oTs[(lc, h)],
                                    start=(h == 0), stop=(h == HPC - 1))
                            if lc == LC - 1:
                                nc.scalar.copy(out=ys[:, i, :], in_=yp)
                            else:
                                nc.vector.tensor_copy(ys[:, i, :], yp)
                        nc.sync.dma_start(
                            out=y_d[b, ebg * 512:(ebg + 1) * 512,
                                    lc * 512:(lc + 1) * 512]
                            .rearrange("(e p) l -> p e l", p=128),
                            in_=ys)

                with (
                    tc.tile_pool(name=f"pss{b}", bufs=3, space="PSUM") as pss,
                    tc.tile_pool(name=f"psa{b}", bufs=2, space="PSUM") as psa,
                    tc.tile_pool(name=f"psr{b}", bufs=1, space="PSUM") as psr,
                    tc.tile_pool(name=f"psy{b}", bufs=2, space="PSUM") as psy,
                ):
                    for lc in range(LC):
                        for h in range(HPC):
                            av = psa.tile([128, 512], F32, name="av", tag="av")
                            rs = psr.tile([1, 512], F32, name="rs", tag="rs")
                            nmb = 4 * lc + 4
                            pend = []

                            def flush(av=av, rs=rs, nmb=nmb, h=h):
                                mb, pt, npr, c0 = pend.pop(0)
                                nc.tensor.matmul(
                                    av[:, c0:512],
                                    lhsT=vv[:, mb, h * 128:(h + 1) * 128],
                                    rhs=pt[:, 0:npr], start=(mb == 0),
                                    stop=(mb == nmb - 1))
                                nc.tensor.matmul(
                                    rs[0:1, c0:512], lhsT=ones_sb,
                                    rhs=pt[:, 0:npr], start=(mb == 0),
                                    stop=(mb == nmb - 1))

                            for mb in range(nmb):
                                l0 = max(lc * 512, mb * 128)
                                npr = lc * 512 + 512 - l0
                                c0 = l0 - lc * 512
                                diag = mb >= 4 * lc
                                st = pss.tile([128, 512], F32, name="st", tag="st")
                                nc.tensor.matmul(
                                    st[:, 0:npr],
                                    lhsT=kT[:, h, mb * 128:(mb + 1) * 128],
                                    rhs=qT[:, h, l0:l0 + npr],
                                    start=True, stop=not diag)
                                if diag:
                                    nc.tensor.matmul(
                                        st[:, 0:128], lhsT=mask_sb, rhs=id_sb,
                                        start=False, stop=True)
                                pt = ptp.tile([128, 512], BF16, name="pt", tag="pt")
                                nc.scalar.activation(
                                    out=pt[:, 0:npr], in_=st[:, 0:npr], func=Exp)
                                pend.append((mb, pt, npr, c0))
                                if len(pend) >= 3:
                                    flush()
                                # slot the previous chunk's out-proj groups
                                # into the last 4 block iterations so the PE
                                # never waits on the normalize chain and the
                                # evac copies spread across the DVE queue
                                if h == 0 and lc > 0 and mb >= nmb - 4:
                                    out_proj_group(lc - 1, mb - (nmb - 4))
                            while pend:
                                flush()

                            rec = smallp.tile([1, 512], F32, name="rec", tag="rec")
                            nc.vector.reciprocal(out=rec, in_=rs)
                            bcs = bcp.tile([128, 512], F32, name="bcs", tag="bcs")
                            nc.gpsimd.partition_broadcast(bcs, rec, channels=128)
                            oTt = otp.tile([128, 512], BF16,
                                           name=f"oT{lc}_{h}", tag=f"oT{lc}_{h}")
                            nc.vector.tensor_mul(oTt, av, bcs)
                            oTs[(lc, h)] = oTt
                    for ebg in range(4):
                        out_proj_group(LC - 1, ebg)
    nc.compile()
    return nc


_NC_CACHE = None


def build_in_maps(x, Wq, bq, Wk, bk, Wv, bv, Wo, bo):
    import ml_dtypes

    BF = ml_dtypes.bfloat16
    x = np.asarray(x, np.float32)
    scale = HD ** (-0.5)

    inv = 1.0 / (BASE ** (np.arange(0, HD, 2, dtype=np.float32) / HD))
    fr = np.outer(inv, np.arange(L, dtype=np.float32))          # [64, L]
    cosf = np.cos(fr).astype(np.float32)
    sinf = np.sin(fr).astype(np.float32)
    cos2 = np.concatenate([cosf, cosf], 0)                      # [128, L]
    sinpm = np.concatenate([-sinf, sinf], 0)                    # [128, L]
    tabs = np.ascontiguousarray(
        np.stack([cos2, sinpm], 1)).astype(BF)                  # [128, 2, L]
    mask = np.where(np.arange(128)[:, None] <= np.arange(128)[None, :],
                    0.0, NEG).astype(np.float32)
    mi = np.ascontiguousarray(
        np.concatenate([mask.T, np.eye(128, dtype=np.float32)], 1)).astype(BF)

    xT = np.transpose(x, (0, 2, 1))                             # [B, E, L]
    xkt = np.ascontiguousarray(
        xT.reshape(B, KT, 128, L)).astype(BF)                   # [B, KT, 128, L]

    Wq = np.asarray(Wq, np.float32)
    Wk = np.asarray(Wk, np.float32)
    Wv = np.asarray(Wv, np.float32)
    Wo = np.asarray(Wo, np.float32)
    bq = np.asarray(bq, np.float32)
    bk = np.asarray(bk, np.float32)
    bv = np.asarray(bv, np.float32)
    bo = np.asarray(bo, np.float32)

    in_maps = []
    for c in range(NCORES):
        cols = slice(c * COLS, (c + 1) * COLS)
        # weights pre-transposed to [128 partition, KT, COLS] so each lands
        # in one long-run DMA
        wq_c = np.ascontiguousarray(
            (Wq[:, cols] * scale).reshape(KT, 128, COLS)
            .transpose(1, 0, 2)).astype(BF)
        wk_c = np.ascontiguousarray(
            Wk[:, cols].reshape(KT, 128, COLS).transpose(1, 0, 2)).astype(BF)
        wv_c = np.ascontiguousarray(
            Wv[:, cols].reshape(KT, 128, COLS).transpose(1, 0, 2)).astype(BF)
        wo_c = np.ascontiguousarray(
            Wo[cols, :].reshape(HPC, 128, E).transpose(1, 0, 2)).astype(BF)
        # biases as [128 partition, (bq h0, bq h1, bk h0, bk h1, then the
        # same four half-swapped for the qsw/ksw evacuations)] f32
        bqh = (bq[cols] * scale).reshape(HPC, 128)
        bkh = bk[cols].reshape(HPC, 128)
        sw = np.r_[np.arange(64, 128), np.arange(0, 64)]
        bias2 = np.ascontiguousarray(np.stack(
            [bqh[0], bqh[1], bkh[0], bkh[1],
             bqh[0][sw], bqh[1][sw], bkh[0][sw], bkh[1][sw]], 1)
        ).astype(np.float32)
        in_maps.append({
            "xkt": xkt,
            "wq": wq_c, "wk": wk_c, "wv": wv_c, "wo": wo_c,
            "bias2": bias2, "tabs": tabs, "mi": mi,
        })
    return in_maps


def kernel(x, Wq, bq, Wk, bk, Wv, bv, Wo, bo):
    global _NC_CACHE
    from concourse.bass_utils import run_bass_kernel_spmd

    in_maps = build_in_maps(x, Wq, bq, Wk, bk, Wv, bv, Wo, bo)
    Wo = np.asarray(Wo, np.float32)
    bv = np.asarray(bv, np.float32)
    bo = np.asarray(bo, np.float32)

    if _NC_CACHE is None:
        _NC_CACHE = _build_program()
    res = run_bass_kernel_spmd(_NC_CACHE, in_maps, list(range(NCORES)))
    acc = np.zeros((B, E, L), np.float64)
    for c in range(NCORES):
        acc += res.results[c]["yT"].astype(np.float32)
    # v-bias folds out of attention (softmax rows sum to 1): out@Wo picks up
    # the constant bv@Wo term, added here in full precision along with bo.
    bias = bo + bv @ Wo
    y = (np.transpose(acc, (0, 2, 1)) + bias).astype(np.float32)
    return y


# revision 3
# speedup vs baseline: 1.0378x; 1.0143x over previous
"""Tensor-parallel MHSA (RoPE + causal attention) for 8 TRN2 NeuronCores, v2.

Sharding: 8-way tensor-parallel over heads (16 heads -> 2 per core), both
batches on every core.  Each core computes q/k/v projections for its 2 heads
(column-parallel), RoPE, causal attention, and a row-parallel slice of the
output projection, producing a full-shape partial y^T in bf16; the host sums
the 8 partials and adds bo + Wo^T bv (the v-bias folds out of attention since
softmax rows sum to 1).

Key structure:
- all matmuls bf16 (same PE rate as fp32r, half the DMA/SBUF of fp32)
- few, large DMAs (HWDGE issue overhead is ~625ns each): weights land in one
  pre-transposed [128, KT, COLS] transfer each, x in 1MB k-pair tiles,
  y in [128, 4eb, 512] groups, constants packed/memset
- QKV runs as three k-outer phases (q, k, v) over 8 single-bank PSUM
  accumulation chains, so each arriving x k-pair immediately feeds 32
  matmuls (hides the DMA ramp) and each stationary weight is reused 8x
  (amortizes LDWEIGHTS)
- each PSUM bank holds exactly ONE accumulation group (zero-region rule):
  both heads' chains in a bank share one start/stop pair
- q/k biases applied during PSUM evacuation (ACT Identity+bias / DVE
  tensor_scalar_add); v bias folds into the host-side output bias
- RoPE on half-rows with bf16 2x DVE tensor_tensor (sign folded into table)
- causal mask added by a tiny PE matmul (maskT^T @ I) inside the scores
  accumulation group
- attention: S^T layout, lookahead-2 block pipeline, exp on ACT -> bf16;
  out-proj for chunk lc-1 emitted inside chunk lc's first head so the PE
  never waits on the softmax-normalize chain
"""
import sys
sys.path.insert(0, "/opt/trn_rl_repo")
import numpy as np

B, L, E = 2, 2048, 2048
HEADS = 16
HD = 128
BASE = 10000.0
NCORES = 8
HPC = HEADS // NCORES      # heads per core = 2
COLS = HPC * HD            # 256 columns of Wq/Wk/Wv per core
KT = E // 128              # 16 k-tiles
LC = L // 512              # 4 l-chunks (attention / out-proj)
SC = L // 256              # 8 sub-chunks (qkv projection)
NEG = -1.0e9


def _build_program():
    import concourse.bass as bass
    import concourse.mybir as mybir
    import concourse.tile as tile
    from concourse import bacc

    F32 = mybir.dt.float32
    F32R = mybir.dt.float32r
    BF16 = mybir.dt.bfloat16
    Exp = mybir.ActivationFunctionType.Exp

    nc = bacc.Bacc()
    x_d = nc.declare_dram_parameter("xkt", [B, KT, 128, L], BF16, isOutput=False)
    wq_d = nc.declare_dram_parameter("wq", [128, KT, COLS], BF16, isOutput=False)
    wk_d = nc.declare_dram_parameter("wk", [128, KT, COLS], BF16, isOutput=False)
    wv_d = nc.declare_dram_parameter("wv", [128, KT, COLS], BF16, isOutput=False)
    wo_d = nc.declare_dram_parameter("wo", [128, HPC, E], BF16, isOutput=False)
    bias_d = nc.declare_dram_parameter("bias2", [128, 4], F32, isOutput=False)
    tabs_d = nc.declare_dram_parameter("tabs", [128, 2, L], BF16, isOutput=False)
    mi_d = nc.declare_dram_parameter("mi", [128, 256], BF16, isOutput=False)
    y_d = nc.declare_dram_parameter("yT", [B, E, L], BF16, isOutput=True)

    with nc.allow_low_precision(reason="bf16 matmuls"), \
         tile.TileContext(nc) as tc:
        with (
            tc.tile_pool(name="fixed", bufs=1) as fixed,
            tc.tile_pool(name="xs", bufs=1) as xs,
            tc.tile_pool(name="qkv", bufs=1) as qkvp,
            tc.tile_pool(name="rope", bufs=2) as rp,
            tc.tile_pool(name="bc", bufs=1) as bcp,
            tc.tile_pool(name="pt", bufs=4) as ptp,
            tc.tile_pool(name="ot", bufs=2) as otp,
            tc.tile_pool(name="yst", bufs=3) as yst,
            tc.tile_pool(name="small", bufs=2) as smallp,
        ):
            # DMA order = consumption order: wq, then x pairs (the q phase
            # tracks x arrival), then wk/wv, then late-use constants.
            wq_sb = fixed.tile([128, KT, COLS], BF16, name="wq", tag="wq")
            nc.sync.dma_start(out=wq_sb[:, 0:8, :], in_=wq_d[:, 0:8, :])
            nc.sync.dma_start(out=wq_sb[:, 8:KT, :], in_=wq_d[:, 8:KT, :])
            xk = [xs.tile([128, 2, L], BF16, name=f"xk{g}", tag=f"xk{g}")
                  for g in range(KT // 2)]
            for g in range(KT // 2):
                nc.sync.dma_start(
                    out=xk[g],
                    in_=x_d[0, 2 * g:2 * g + 2].rearrange("k p l -> p k l"))
            wk_sb = fixed.tile([128, KT, COLS], BF16, name="wk", tag="wk")
            nc.sync.dma_start(out=wk_sb, in_=wk_d[:, :, :])
            wv_sb = fixed.tile([128, KT, COLS], BF16, name="wv", tag="wv")
            nc.sync.dma_start(out=wv_sb, in_=wv_d[:, :, :])
            bias_sb = fixed.tile([128, 4], F32, name="bias2", tag="bias2")
            nc.sync.dma_start(out=bias_sb, in_=bias_d[:, :])
            tabs_sb = fixed.tile([128, 2, L], BF16, name="tabs", tag="tabs")
            nc.sync.dma_start(out=tabs_sb, in_=tabs_d[:, :, :])
            mi_sb = fixed.tile([128, 256], BF16, name="mi", tag="mi")
            nc.sync.dma_start(out=mi_sb, in_=mi_d[:, :])
            wo_sb = fixed.tile([128, HPC, E], BF16, name="wo", tag="wo")
            nc.sync.dma_start(out=wo_sb, in_=wo_d[:, :, :])

            cos_sb = tabs_sb[:, 0, :]
            sin_sb = tabs_sb[:, 1, :]
            mask_sb = mi_sb[:, 0:128]
            id_sb = mi_sb[:, 128:256]
            ones_sb = fixed.tile([128, 1], BF16, name="onesb", tag="onesb")
            nc.vector.memset(ones_sb, 1.0)

            def xkt_ap(k):
                return xk[k // 2][:, k % 2, :]

            for b in range(B):
                if b > 0:
                    for g in range(KT // 2):
                        nc.sync.dma_start(
                            out=xk[g],
                            in_=x_d[b, 2 * g:2 * g + 2]
                            .rearrange("k p l -> p k l"))

                qT = qkvp.tile([128, HPC, L], BF16, name="qT", tag="qT")
                kT = qkvp.tile([128, HPC, L], BF16, name="kT", tag="kT")
                qraw = qkvp.tile([128, HPC, L], BF16, name="qraw", tag="qraw")
                kraw = qkvp.tile([128, HPC, L], BF16, name="kraw", tag="kraw")
                qsw = qkvp.tile([128, HPC, L], BF16, name="qsw", tag="qsw")
                ksw = qkvp.tile([128, HPC, L], BF16, name="ksw", tag="ksw")
                vv = qkvp.tile([128, 16, COLS], BF16, name="vv", tag="vv")

                def rope_chunk(hc):
                    ch = slice(hc * 1024, hc * 1024 + 1024)
                    for src, ssw in ((qraw, qsw), (kraw, ksw)):
                        # half-swap staged by DMA (engines cannot cross base
                        # partitions between SBUF operands; DMA can)
                        nc.sync.dma_start(out=ssw[0:64, :, ch],
                                          in_=src[64:128, :, ch])
                        nc.sync.dma_start(out=ssw[64:128, :, ch],
                                          in_=src[0:64, :, ch])
                    for h in range(HPC):
                        for src, ssw, dst in ((qraw, qsw, qT), (kraw, ksw, kT)):
                            t1 = rp.tile([128, 1024], BF16, name="t1", tag="t1")
                            nc.vector.tensor_mul(t1, ssw[:, h, ch], sin_sb[:, ch])
                            t2 = rp.tile([128, 1024], BF16, name="t2", tag="t2")
                            nc.vector.tensor_mul(t2, src[:, h, ch], cos_sb[:, ch])
                            nc.vector.tensor_add(dst[:, h, ch], t1, t2)

                # ---------- QKV projection over 8 PSUM banks ----------
                # q phase is k-outer so each arriving x k-pair feeds all 8
                # chains (tracks the DMA ramp); k and v phases run sc-outer
                # (x already resident) so their evacuations spread across the
                # phase instead of bunching at its end.
                with tc.tile_pool(name=f"psq{b}", bufs=1, space="PSUM") as psq:
                    qps = [psq.tile([128, HPC, 256], F32,
                                    name=f"ps{sc}", tag=f"ps{sc}")
                           for sc in range(SC)]
                    for k in range(KT):
                        for h in range(HPC):
                            for sc in range(SC):
                                nc.tensor.matmul(
                                    qps[sc][:, h, :],
                                    lhsT=wq_sb[:, k, h * 128:(h + 1) * 128],
                                    rhs=xkt_ap(k)[:, sc * 256:(sc + 1) * 256],
                                    start=(k == 0 and h == 0),
                                    stop=(k == KT - 1 and h == HPC - 1))
                    for sc in range(SC):
                        sl = slice(sc * 256, (sc + 1) * 256)
                        nc.scalar.add(out=qraw[:, 0, sl], in_=qps[sc][:, 0, :],
                                      add=bias_sb[:, 0:1])
                        nc.vector.tensor_scalar_add(
                            out=qraw[:, 1, sl], in0=qps[sc][:, 1, :],
                            scalar1=bias_sb[:, 1:2])

                    for sc in range(SC):
                        kps = psq.tile([128, HPC, 256], F32,
                                       name=f"ps{sc}", tag=f"ps{sc}")
                        sl = slice(sc * 256, (sc + 1) * 256)
                        for k in range(KT):
                            for h in range(HPC):
                                nc.tensor.matmul(
                                    kps[:, h, :],
                                    lhsT=wk_sb[:, k, h * 128:(h + 1) * 128],
                                    rhs=xkt_ap(k)[:, sl],
                                    start=(k == 0 and h == 0),
                                    stop=(k == KT - 1 and h == HPC - 1))
                        nc.scalar.add(out=kraw[:, 0, sl], in_=kps[:, 0, :],
                                      add=bias_sb[:, 2:3])
                        nc.vector.tensor_scalar_add(
                            out=kraw[:, 1, sl], in0=kps[:, 1, :],
                            scalar1=bias_sb[:, 3:4])
                        if sc == 3:
                            rope_chunk(0)
                    rope_chunk(1)

                    # v phase: V in token-major (x slices stationary)
                    for sc in range(SC):
                        vps = psq.tile([128, 2, 256], F32,
                                       name=f"ps{sc}", tag=f"ps{sc}")
                        for k in range(KT):
                            for i in range(2):
                                nc.tensor.matmul(
                                    vps[:, i, :],
                                    lhsT=xkt_ap(k)[:, sc * 256 + i * 128:
                                                   sc * 256 + (i + 1) * 128],
                                    rhs=wv_sb[:, k, :],
                                    start=(k == 0 and i == 0),
                                    stop=(k == KT - 1 and i == 1))
                        nc.scalar.copy(out=vv[:, 2 * sc:2 * sc + 2, :],
                                       in_=vps)

                # ---------- causal attention + interleaved out-proj ----------
                oTs = {}

                def out_proj_group(lc, ebg, gsz=4):
                    if True:
                        ys = yst.tile([128, 4, 512], BF16, name="ys", tag="ys")
                        for i in range(gsz):
                            eb = ebg * gsz + i
                            yp = psy.tile([128, 512], F32, name="yp", tag="yp")
                            for h in range(HPC):
                                nc.tensor.matmul(
                                    yp,
                                    lhsT=wo_sb[:, h, eb * 128:(eb + 1) * 128],
                                    rhs=oTs[(lc, h)],
                                    start=(h == 0), stop=(h == HPC - 1))
                            if lc == LC - 1 and i % 2 == 0:
                                nc.scalar.copy(out=ys[:, i, :], in_=yp)
                            else:
                                nc.vector.tensor_copy(ys[:, i, :], yp)
                        nc.sync.dma_start(
                            out=y_d[b, ebg * gsz * 128:(ebg + 1) * gsz * 128,
                                    lc * 512:(lc + 1) * 512]
                            .rearrange("(e p) l -> p e l", p=128),
                            in_=ys[:, 0:gsz, :])

                with (
                    tc.tile_pool(name=f"pss{b}", bufs=3, space="PSUM") as pss,
                    tc.tile_pool(name=f"psa{b}", bufs=2, space="PSUM") as psa,
                    tc.tile_pool(name=f"psr{b}", bufs=1, space="PSUM") as psr,
                    tc.tile_pool(name=f"psy{b}", bufs=2, space="PSUM") as psy,
                ):
                    for lc in range(LC):
                        for h in range(HPC):
                            av = psa.tile([128, 512], F32, name="av", tag="av")
                            rs = psr.tile([1, 512], F32, name="rs", tag="rs")
                            nmb = 4 * lc + 4
                            pend = []

                            def flush(av=av, rs=rs, nmb=nmb, h=h):
                                mb, pt, npr, c0 = pend.pop(0)
                                nc.tensor.matmul(
                                    av[:, c0:512],
                                    lhsT=vv[:, mb, h * 128:(h + 1) * 128],
                                    rhs=pt[:, 0:npr], start=(mb == 0),
                                    stop=(mb == nmb - 1))
                                nc.tensor.matmul(
                                    rs[0:1, c0:512], lhsT=ones_sb,
                                    rhs=pt[:, 0:npr], start=(mb == 0),
                                    stop=(mb == nmb - 1))

                            for mb in range(nmb):
                                l0 = max(lc * 512, mb * 128)
                                npr = lc * 512 + 512 - l0
                                c0 = l0 - lc * 512
                                diag = mb >= 4 * lc
                                st = pss.tile([128, 512], F32, name="st", tag="st")
                                nc.tensor.matmul(
                                    st[:, 0:npr],
                                    lhsT=kT[:, h, mb * 128:(mb + 1) * 128],
                                    rhs=qT[:, h, l0:l0 + npr],
                                    start=True, stop=not diag)
                                if diag:
                                    nc.tensor.matmul(
                                        st[:, 0:128], lhsT=mask_sb, rhs=id_sb,
                                        start=False, stop=True)
                                pt = ptp.tile([128, 512], BF16, name="pt", tag="pt")
                                nc.scalar.activation(
                                    out=pt[:, 0:npr], in_=st[:, 0:npr], func=Exp)
                                pend.append((mb, pt, npr, c0))
                                if len(pend) >= 3:
                                    flush()
                                # slot the previous chunk's out-proj groups
                                # into the last 4 block iterations so the PE
                                # never waits on the normalize chain and the
                                # evac copies spread across the DVE queue
                                if h == 0 and lc > 0 and mb >= nmb - 4:
                                    out_proj_group(lc - 1, mb - (nmb - 4))
                            while pend:
                                flush()

                            rec = smallp.tile([1, 512], F32, name="rec", tag="rec")
                            nc.vector.reciprocal(out=rec, in_=rs)
                            bcs = bcp.tile([128, 512], F32, name="bcs", tag="bcs")
                            nc.gpsimd.partition_broadcast(bcs, rec, channels=128)
                            oTt = otp.tile([128, 512], BF16,
                                           name=f"oT{lc}_{h}", tag=f"oT{lc}_{h}")
                            nc.vector.tensor_mul(oTt, av, bcs)
                            oTs[(lc, h)] = oTt
                    for ebg in range(4):
                        out_proj_group(LC - 1, ebg)
    nc.compile()
    return nc


_NC_CACHE = None


def build_in_maps(x, Wq, bq, Wk, bk, Wv, bv, Wo, bo):
    import ml_dtypes

    BF = ml_dtypes.bfloat16
    x = np.asarray(x, np.float32)
    scale = HD ** (-0.5)

    inv = 1.0 / (BASE ** (np.arange(0, HD, 2, dtype=np.float32) / HD))
    fr = np.outer(inv, np.arange(L, dtype=np.float32))          # [64, L]
    cosf = np.cos(fr).astype(np.float32)
    sinf = np.sin(fr).astype(np.float32)
    cos2 = np.concatenate([cosf, cosf], 0)                      # [128, L]
    sinpm = np.concatenate([-sinf, sinf], 0)                    # [128, L]
    tabs = np.ascontiguousarray(
        np.stack([cos2, sinpm], 1)).astype(BF)                  # [128, 2, L]
    mask = np.where(np.arange(128)[:, None] <= np.arange(128)[None, :],
                    0.0, NEG).astype(np.float32)
    mi = np.ascontiguousarray(
        np.concatenate([mask.T, np.eye(128, dtype=np.float32)], 1)).astype(BF)

    xT = np.transpose(x, (0, 2, 1))                             # [B, E, L]
    xkt = np.ascontiguousarray(
        xT.reshape(B, KT, 128, L)).astype(BF)                   # [B, KT, 128, L]

    Wq = np.asarray(Wq, np.float32)
    Wk = np.asarray(Wk, np.float32)
    Wv = np.asarray(Wv, np.float32)
    Wo = np.asarray(Wo, np.float32)
    bq = np.asarray(bq, np.float32)
    bk = np.asarray(bk, np.float32)
    bv = np.asarray(bv, np.float32)
    bo = np.asarray(bo, np.float32)

    in_maps = []
    for c in range(NCORES):
        cols = slice(c * COLS, (c + 1) * COLS)
        # weights pre-transposed to [128 partition, KT, COLS] so each lands
        # in one long-run DMA
        wq_c = np.ascontiguousarray(
            (Wq[:, cols] * scale).reshape(KT, 128, COLS)
            .transpose(1, 0, 2)).astype(BF)
        wk_c = np.ascontiguousarray(
            Wk[:, cols].reshape(KT, 128, COLS).transpose(1, 0, 2)).astype(BF)
        wv_c = np.ascontiguousarray(
            Wv[:, cols].reshape(KT, 128, COLS).transpose(1, 0, 2)).astype(BF)
        wo_c = np.ascontiguousarray(
            Wo[cols, :].reshape(HPC, 128, E).transpose(1, 0, 2)).astype(BF)
        # biases as [128 partition, (bq h0, bq h1, bk h0, bk h1, then the
        # same four half-swapped for the qsw/ksw evacuations)] f32
        bqh = (bq[cols] * scale).reshape(HPC, 128)
        bkh = bk[cols].reshape(HPC, 128)
        bias2 = np.ascontiguousarray(np.stack(
            [bqh[0], bqh[1], bkh[0], bkh[1]], 1)).astype(np.float32)
        in_maps.append({
            "xkt": xkt,
            "wq": wq_c, "wk": wk_c, "wv": wv_c, "wo": wo_c,
            "bias2": bias2, "tabs": tabs, "mi": mi,
        })
    return in_maps


def kernel(x, Wq, bq, Wk, bk, Wv, bv, Wo, bo):
    global _NC_CACHE
    from concourse.bass_utils import run_bass_kernel_spmd

    in_maps = build_in_maps(x, Wq, bq, Wk, bk, Wv, bv, Wo, bo)
    Wo = np.asarray(Wo, np.float32)
    bv = np.asarray(bv, np.float32)
    bo = np.asarray(bo, np.float32)

    if _NC_CACHE is None:
        _NC_CACHE = _build_program()
    res = run_bass_kernel_spmd(_NC_CACHE, in_maps, list(range(NCORES)))
    acc = np.zeros((B, E, L), np.float64)
    for c in range(NCORES):
        acc += res.results[c]["yT"].astype(np.float32)
    # v-bias folds out of attention (softmax rows sum to 1): out@Wo picks up
    # the constant bv@Wo term, added here in full precision along with bo.
    bias = bo + bv @ Wo
    y = (np.transpose(acc, (0, 2, 1)) + bias).astype(np.float32)
    return y
